# revision 20
# baseline (speedup 1.0000x reference)
"""Trainium2 Bass kernel for ContrastiveHessianCalculator GGN-diagonal.

Math (see docstring of the reference):
  out = concat([W1d.flat, b1d, W2d.flat, b2d])   # [164416]
  c_i = sum_o W2[o,i]^2
  For a pair batch (ia, ib):
    h = tanh(x @ W1.T + b1); d = 1 - h^2 (per side a/b)
    W1d[i,j] = c_i * sum_p (da^2 xa_j^2 - 2 da db xa_j xb_j + db^2 xb_j^2)
    b1d[i]   = c_i * sum_p (da - db)^2
    W2d[o,i] = sum_p (ha - hb)^2   (same for every o);  b2d = 0
  out = pos-pairs - neg-pairs.

The p-sum is a matmul:  W1d_raw = U^T @ V  with U k-tiles
[da^2, -2dadb, db^2, hd] (pos and neg) and V k-tiles being the matching
x-products (negated for neg).  b1d comes from an extra all-{+1,-1} column
of V; hd from a one-hot column.  Sharding: data-parallel over the pair
dim P across 8 cores (P/8=128 pairs each -> every tile is exactly one
128-partition tile), ReduceScatter of the [128,4,258] partial, identical
final assembly on the host.

Perf notes (cost-model driven):
  - all PE matmuls use bf16 operands (1 cyc/row vs 4 for fp32); the bias
    add rides the z PSUM group as a K=1 matmul with fp32r operands.
  - elementwise U/V math in bf16 on DVE (2x packed mode), transcendental
    and (1-x)^2 steps on ACT (dtype-agnostic cost), memsets/small copies
    on Pool.
  - one merged indirect gather [128,4,256] (SWDGE prep is ~1us fixed per
    DMA), one W1 load, one partial store; collective payload in bf16.
"""

import numpy as np

import concourse.bass as bass
import concourse.tile as tile
from concourse import bacc, bass_utils, mybir
from concourse.masks import make_identity

F32 = mybir.dt.float32
F32R = mybir.dt.float32r
BF16 = mybir.dt.bfloat16
I32 = mybir.dt.int32
AF = mybir.ActivationFunctionType
ALU = mybir.AluOpType

N, D, H, O, P = 50000, 256, 512, 64, 1024
NCORES = 8
PP = P // NCORES          # 128 pairs per core per pos/neg block
HC = H // 128             # 4 h-chunks
DC = D // 128             # 2 d-chunks
NPARAM = H * D + H + O * H + O  # 164416
VW = D + 2                # V tile width: 256 data + b1-ones col + hd one-hot col
SH = 128 // NCORES        # ReduceScatter shard rows per core

_CACHE = {}


def _build_program():
    nc = bacc.Bacc(
        "TRN2",
        debug=False,
        enable_asserts=False,
        target_bir_lowering=False,
        num_devices=NCORES,
    )

    x_d = nc.dram_tensor("x", [N, D], F32, kind="ExternalInput").ap()
    w1_d = nc.dram_tensor("W1", [H, D], F32, kind="ExternalInput").ap()
    b1_d = nc.dram_tensor("b1r", [1, H], F32, kind="ExternalInput").ap()
    w2_d = nc.dram_tensor("W2", [O, H], F32, kind="ExternalInput").ap()
    idx_d = nc.dram_tensor("idx", [PP, 4], I32, kind="ExternalInput").ap()
    # per-core output: this core's ReduceScatter shard of the summed
    # [128, HC, VW] partial (W1d rows + b1d col + hd col), in bf16
    shard_d = nc.dram_tensor("shard", [SH, HC, VW], BF16, kind="ExternalOutput").ap()

    with tile.TileContext(nc) as tc:
        _body(tc, x_d, w1_d, b1_d, w2_d, idx_d, shard_d)
    nc.compile()
    return nc


def _body(tc, x_d, w1_d, b1_d, w2_d, idx_d, shard_d):
    nc = tc.nc
    from contextlib import ExitStack

    ctx = ExitStack()
    singles = ctx.enter_context(tc.tile_pool(name="singles", bufs=1))
    work = ctx.enter_context(tc.tile_pool(name="work", bufs=1))
    ps_z = ctx.enter_context(tc.tile_pool(name="ps_z", bufs=2, space="PSUM"))
    ps_t = ctx.enter_context(tc.tile_pool(name="ps_t", bufs=2, space="PSUM"))
    ps_w = ctx.enter_context(tc.tile_pool(name="ps_w", bufs=1, space="PSUM"))
    dram = ctx.enter_context(tc.tile_pool(name="dram", bufs=1, space="DRAM"))

    # ---- input DMAs first: idx gates the gather which gates everything ----
    idx_sb = singles.tile([PP, 4], I32)
    nc.sync.dma_start(out=idx_sb[:], in_=idx_d[:])
    xg = work.tile([128, 4, D], F32, name="xg")
    for j in range(4):
        nc.gpsimd.indirect_dma_start(
            out=xg[:, j, :],
            out_offset=None,
            in_=x_d[:],
            in_offset=bass.IndirectOffsetOnAxis(ap=idx_sb[:, j : j + 1], axis=0),
        )
    # W1 as one DMA into [128, HC, D]: partition p holds W1[hc*128+p, :]
    w1_sb = singles.tile([128, HC, D], F32)
    w1_r = w1_d.rearrange("(a b) c -> b a c", a=HC)
    for dc in range(DC):
        nc.scalar.dma_start(
            out=w1_sb[:, :, dc * 128 : (dc + 1) * 128],
            in_=w1_r[:, :, dc * 128 : (dc + 1) * 128],
        )
    ident = singles.tile([128, 128], F32)
    make_identity(nc, ident[:])
    b1row = singles.tile([1, H], F32)
    b1_bf = singles.tile([1, H], BF16)
    nc.scalar.dma_start(out=b1row[:], in_=b1_d[:])
    nc.vector.tensor_copy(out=b1_bf[:], in_=b1row[:])
    w2_sb = singles.tile([O, H], F32)
    nc.sync.dma_start(out=w2_sb[:], in_=w2_d[:])
    ones_r = singles.tile([1, 128], BF16)
    nc.vector.memset(ones_r[:], 1.0)
    ones64 = singles.tile([O, 1], F32)
    nc.gpsimd.memset(ones64[:], 1.0)

    # ---- V constant columns (independent of data; emit early, Pool) ----
    # V layout: [128, 6, D+1]; k = blk*3 + {0:vaa, 1:vab, 2:vbb}; the hd
    # column comes from tiny N=1 matmuls instead of one-hot V tiles
    v_all = work.tile([128, 6, D + 1], BF16, name="v_all")
    for blk, sgn in ((0, 1.0), (1, -1.0)):
        k0 = 3 * blk
        nc.gpsimd.memset(v_all[:, k0 : k0 + 3, D : D + 1], sgn)   # b1d col
    sgn_one = singles.tile([128, 2], BF16)
    nc.gpsimd.memset(sgn_one[:, 0:1], 1.0)
    nc.gpsimd.memset(sgn_one[:, 1:2], -1.0)

    # ---- PE p-state warm-up: dead ident transposes during DMA latency ----
    for w in range(5):
        wpt = ps_t.tile([128, 512], F32, tag="tp")
        for q in range(4):
            nc.tensor.transpose(
                wpt[:, q * 128 : (q + 1) * 128], ident[:], ident[:]
            )

    # ---- W1 transposes: per dc a [128,512] PSUM tile, 4 PE transposes,
    #      one copy (downcast) to bf16 ----
    w1t = singles.tile([128, DC, H], BF16)
    for dc in range(DC):
        pt = ps_t.tile([128, 512], F32, tag="tp")
        for hc in range(HC):
            nc.tensor.transpose(
                pt[:, hc * 128 : (hc + 1) * 128],
                w1_sb[:, hc, dc * 128 : (dc + 1) * 128],
                ident[:],
            )
        if dc == 0:
            nc.scalar.copy(out=w1t[:, dc, :], in_=pt[:])
        else:
            nc.vector.tensor_copy(out=w1t[:, dc, :], in_=pt[:])

    # ---- xg transposes, grouped per gather j so z_j starts as soon as
    #      gather j lands (not after all four) ----
    xgt = work.tile([128, 4, DC, 128], BF16, name="xgt")
    for j in range(4):
        pt = ps_t.tile([128, 512], F32, tag="tp")
        for dc in range(DC):
            nc.tensor.transpose(
                pt[:, dc * 128 : (dc + 1) * 128],
                xg[:, j, dc * 128 : (dc + 1) * 128],
                ident[:],
            )
        if j % 2 == 0:
            nc.vector.tensor_copy(out=xgt[:, j, :, :], in_=pt[:, :256])
        else:
            nc.scalar.copy(out=xgt[:, j, :, :], in_=pt[:, :256])

    # ---- z = xg @ W1.T + b1 (bias as K=1 matmul); ha = tanh(z) in bf16 ----
    ha = work.tile([128, 4, H], BF16, name="ha")
    for j in range(4):
        zp = ps_z.tile([128, H], F32, tag="z")
        for dc in range(DC):
            nc.tensor.matmul(
                zp[:], lhsT=xgt[:, j, dc, :], rhs=w1t[:, dc, :],
                start=(dc == 0), stop=False,
            )
        nc.tensor.matmul(
            zp[:], lhsT=ones_r[:], rhs=b1_bf[:],
            start=False, stop=True,
        )
        nc.scalar.activation(out=ha[:, j, :], in_=zp[:], func=AF.Tanh)

    # ---- U tiles [128, 8, H] bf16; k = blk*4 + {0:daSq, 1:m2dadb, 2:dbSq, 3:hd}
    #      daSq/dbSq as one strided ACT op per block: Square(1 - hsq) ----
    u_all = work.tile([128, 2, 2, 2, H], BF16, name="u_all")  # [blk, pp, t2]
    hsq = work.tile([128, 4, H], BF16, name="hsq")
    da_m2 = work.tile([128, 2, H], BF16, name="da_m2")
    dbv = work.tile([128, 2, H], BF16, name="dbv")
    hdd = work.tile([128, 2, H], BF16, name="hdd")
    for blk in range(2):
        j0 = 2 * blk
        for s in range(2):
            nc.vector.tensor_mul(
                hsq[:, j0 + s, :], ha[:, j0 + s, :], ha[:, j0 + s, :]
            )
            nc.scalar.activation(
                out=u_all[:, blk, s, 0, :], in_=hsq[:, j0 + s, :],
                func=AF.Square, bias=1.0, scale=-1.0,
            )
        # da_m2 = -2*(1-ha^2) = 2*hsq - 2 ; dbv = 1 - hsq_b
        nc.vector.tensor_scalar(
            da_m2[:, blk, :], hsq[:, j0, :], 2.0, -2.0, ALU.mult, ALU.add
        )
        nc.vector.tensor_scalar(
            dbv[:, blk, :], hsq[:, j0 + 1, :], -1.0, 1.0, ALU.mult, ALU.add
        )
        nc.vector.tensor_mul(
            u_all[:, blk, 0, 1, :], da_m2[:, blk, :], dbv[:, blk, :]
        )
        nc.vector.tensor_sub(hdd[:, blk, :], ha[:, j0, :], ha[:, j0 + 1, :])
        nc.vector.tensor_mul(u_all[:, blk, 1, 1, :], hdd[:, blk, :], hdd[:, blk, :])

    # ---- V data columns ----
    # pos block: vaa = g0^2, vbb = g1^2 (two ACT squares), vab = g0*g1
    nc.scalar.activation(out=v_all[:, 0, :D], in_=xg[:, 0, :], func=AF.Square)
    nc.scalar.activation(out=v_all[:, 2, :D], in_=xg[:, 1, :], func=AF.Square)
    nc.vector.tensor_mul(v_all[:, 1, :D], xg[:, 2 * 0, :], xg[:, 1, :])
    # neg block: squares+product into tmp, then negate (bf16 2x) into V
    tmpn = work.tile([128, 2, 2, D], BF16, name="tmpn")
    nc.scalar.activation(out=tmpn[:, :, 0, :], in_=xg[:, 2:4, :], func=AF.Square)
    nc.vector.tensor_mul(tmpn[:, 0, 1, :], xg[:, 2, :], xg[:, 3, :])
    # v k4 = -g2^2, k5 = -g2*g3 (tmpn[0,0],tmpn[0,1] contiguous)
    nc.vector.tensor_scalar_mul(v_all[:, 3:5, :D], tmpn[:, 0, :, :], -1.0)
    nc.vector.tensor_scalar_mul(v_all[:, 5, :D], tmpn[:, 1, 0, :], -1.0)

    # ---- c = colsum(W2^2) as per-partition chunks ----
    w2sq = singles.tile([O, H], F32)
    nc.scalar.activation(out=w2sq[:], in_=w2_sb[:], func=AF.Square)
    c_sb = singles.tile([128, HC], F32)
    for hc in range(HC):
        cp = ps_t.tile([128, 1], F32, tag="tp", name="cp")
        nc.tensor.matmul(
            cp[:], lhsT=w2sq[:, hc * 128 : (hc + 1) * 128], rhs=ones64[:],
            start=True, stop=True,
        )
        nc.scalar.copy(out=c_sb[:, hc : hc + 1], in_=cp[:])

    # ---- big matmul, k-outer so each k fires as soon as its U/V are ready ----
    wp = [ps_w.tile([128, VW], F32, tag=f"wp{hc}", name=f"wp{hc}") for hc in range(HC)]
    UK = [(0, 0, 0), (0, 0, 1), (0, 1, 0), (1, 0, 0), (1, 0, 1), (1, 1, 0)]
    for k in range(6):
        blk, pp, t2 = UK[k]
        for hc in range(HC):
            nc.tensor.matmul(
                wp[hc][:, : D + 1],
                lhsT=u_all[:, blk, pp, t2, hc * 128 : (hc + 1) * 128],
                rhs=v_all[:, k, :],
                start=(k == 0), stop=(k == 5),
            )
    # hd column: sum_p hd[p,h] * (+1|-1) via N=1 matmuls per hc
    hp = []
    for hc in range(HC):
        h = ps_t.tile([128, 1], F32, tag="tp", name=f"hp{hc}")
        for blk in range(2):
            nc.tensor.matmul(
                h[:],
                lhsT=u_all[:, blk, 1, 1, hc * 128 : (hc + 1) * 128],
                rhs=sgn_one[:, blk : blk + 1],
                start=(blk == 0), stop=(blk == 1),
            )
        hp.append(h)

    # ---- c post-scale into bf16 partial; hd col copied raw (Pool) ----
    partial = work.tile([128, HC, VW], BF16)
    for hc in range(HC):
        if hc % 2 == 0:
            nc.vector.tensor_scalar_mul(
                partial[:, hc, : D + 1], wp[hc][:, : D + 1], c_sb[:, hc : hc + 1]
            )
        else:
            nc.scalar.activation(
                out=partial[:, hc, : D + 1], in_=wp[hc][:, : D + 1],
                func=AF.Copy, scale=c_sb[:, hc : hc + 1],
            )
        if hc % 2 == 0:
            nc.scalar.copy(out=partial[:, hc, D + 1 : VW], in_=hp[hc][:])
        else:
            nc.vector.tensor_copy(out=partial[:, hc, D + 1 : VW], in_=hp[hc][:])

    # ---- ReduceScatter over the 8 cores straight into the output shard ----
    cc_in = dram.tile([128, HC, VW], BF16)
    nc.sync.dma_start(out=cc_in[:], in_=partial[:])
    rs_out = dram.tile([SH, HC, VW], BF16)
    nc.gpsimd.collective_compute(
        "ReduceScatter",
        ALU.add,
        replica_groups=[list(range(NCORES))],
        ins=[cc_in.opt()],
        outs=[rs_out.opt()],
    )
    nc.sync.dma_start(out=shard_d[:], in_=rs_out[:])
    ctx.close()


def _get_program():
    if "nc" not in _CACHE:
        _CACHE["nc"] = _build_program()
    return _CACHE["nc"]


def kernel(**inputs):
    x = np.ascontiguousarray(np.asarray(inputs["x"], dtype=np.float32))
    W1 = np.ascontiguousarray(np.asarray(inputs["W1"], dtype=np.float32))
    b1 = np.ascontiguousarray(
        np.asarray(inputs["b1"], dtype=np.float32).reshape(1, H)
    )
    W2 = np.ascontiguousarray(np.asarray(inputs["W2"], dtype=np.float32))
    iap = np.asarray(inputs["ap"], dtype=np.int32)
    ip = np.asarray(inputs["p"], dtype=np.int32)
    ian = np.asarray(inputs["an"], dtype=np.int32)
    inn = np.asarray(inputs["n"], dtype=np.int32)

    nc = _get_program()
    in_maps = []
    for i in range(NCORES):
        s = slice(i * PP, (i + 1) * PP)
        idx = np.ascontiguousarray(
            np.stack([iap[s], ip[s], ian[s], inn[s]], axis=1).astype(np.int32)
        )
        in_maps.append({"x": x, "W1": W1, "b1r": b1, "W2": W2, "idx": idx})

    res = bass_utils.run_bass_kernel_spmd(
        nc, in_maps, core_ids=list(range(NCORES))
    )
    return _assemble([res.results[c] for c in range(NCORES)])


def _assemble(per_core):
    """Pure gather/unshard: concatenate the ReduceScatter shards and the
    device-computed W2d/b2d tail into the full [164416] output."""
    shards = np.stack(
        [np.asarray(per_core[c]["shard"], dtype=np.float32) for c in range(NCORES)]
    )  # [8, SH, HC, VW]
    red = shards.transpose(2, 0, 1, 3).reshape(H, VW)  # h = hc*128 + SH*c + q
    out = np.empty(NPARAM, np.float32)
    out[0 : H * D] = red[:, :D].reshape(-1)
    out[H * D : H * D + H] = red[:, D]
    base = H * D + H
    out[base : base + O * H] = np.tile(red[:, D + 1], O)  # W2d rows all equal hd
    out[base + O * H :] = 0.0  # b2d is exactly zero
    return out
